# revision 14
# baseline (speedup 1.0000x reference)
"""CurvSelfAttention Trainium2 kernel — 8-core SPMD, head/tensor-parallel.

Sharding: each of the 8 cores owns 2 of the 16 heads (columns
[128c, 128c+128) of Wq/Wk/Wv/Ws and of the output), and computes both
batches for its heads.  Inside a core:

  phase 1 (projections): X [4096,1024] is streamed in 512-token chunks,
  PE-transposed to X^T tiles, and multiplied with the core's weight
  column slices to produce Q^T,K^T,s^T [128, 4096] (dim-on-partition,
  2 heads stacked) and V^T (then PE-transposed to V [tok,128]).
  s = sigmoid(XWs + bs) + 0.5 is folded into Q^T and K^T (with the
  1/sqrt(D) = 1/8 attention scale folded into Q^T).  The sigmoid is
  computed as 1/(1+exp(-x)) with the exp on ACT so the ACT engine only
  ever needs the Exp function table (a table switch costs ~2.7us).

  phase 2 (attention): for each (batch, 512-query chunk), loop over
  128-token key blocks: scores^T [k, q] for BOTH heads land in one
  [128,1024] PSUM tile via two row-tiled (K=64) matmuls (concurrent in
  the PE array), one exp instruction on ACT covers both heads (no
  max-subtraction: |scores| < ~3 by construction), then per head
  ctx~^T [65, q] += [V | 1].T @ exp(scores^T), accumulating the
  softmax numerator AND denominator (ones column) in one matmul.
  Epilogue (deferred into the next unit's loop): PE-transpose ctx~^T,
  multiply by the reciprocal of the denominator row, DMA out.

The attention phase is ACT-bound (the exp stream), so batch 1's
projection work and the attention epilogues are emitted interleaved
into the attention loops to fill the PE gaps.

Matmul operands are float32r (full PE rate at N=512; true fp32 is 4x
slower).  Walrus requires fp32r-consumed tiles to be produced as
fp32r, so those tiles (and the weight DRAM params) are declared
float32r — numpy still sees float32.
"""

from collections import deque

import numpy as np

B = 2
S = 2048
H = 16
D = 64
DM = H * D            # 1024
T = B * S             # 4096
N_CORES = 8
HPC = H // N_CORES    # heads per core
CW = HPC * D          # 128 output columns per core
TC = 512              # phase-1 token chunk
NTC = T // TC         # 8
QC = 512              # query chunk
NQC = S // QC         # 4
KB = 128              # key block
NKB = S // KB         # 16
NHB = DM // 128       # 8 hidden blocks

_CACHE = {}


def _build_nc(repeats=1):
    from contextlib import ExitStack, nullcontext

    import concourse.bacc as bacc
    import concourse.mybir as mybir
    import concourse.tile as tile
    from concourse.masks import make_identity

    f32 = mybir.dt.float32
    f32r = mybir.dt.float32r
    AF = mybir.ActivationFunctionType
    Alu = mybir.AluOpType

    nc = bacc.Bacc("TRN2", target_bir_lowering=False, debug=False,
                   num_devices=N_CORES)

    x = nc.declare_dram_parameter("x", [T, DM], f32r, isOutput=False)
    wq = nc.declare_dram_parameter("wq", [DM, CW], f32r, isOutput=False)
    wk = nc.declare_dram_parameter("wk", [DM, CW], f32r, isOutput=False)
    wv = nc.declare_dram_parameter("wv", [DM, CW], f32r, isOutput=False)
    ws = nc.declare_dram_parameter("ws", [DM, CW], f32r, isOutput=False)
    bq = nc.declare_dram_parameter("bq", [CW], f32, isOutput=False)
    bk = nc.declare_dram_parameter("bk", [CW], f32, isOutput=False)
    bv = nc.declare_dram_parameter("bv", [CW], f32, isOutput=False)
    bs = nc.declare_dram_parameter("bs", [CW], f32, isOutput=False)
    out = nc.declare_dram_parameter("out", [T, CW], f32, isOutput=True)

    with tile.TileContext(nc) as tc, ExitStack() as ctx:
        # repeats>1 wraps the whole body in a HW loop — used only by the
        # timing harness to amortize the ~100 ms axon dispatch floor.
        if repeats > 1:
            ctx.enter_context(tc.For_i(0, repeats, 1))
        persist = ctx.enter_context(tc.tile_pool(name="persist", bufs=1))

        w_sb = persist.tile([128, 4, NHB, CW], f32r)
        bias_sb = persist.tile([CW, 4], f32)
        neg_bs = persist.tile([CW, 1], f32)
        ident = persist.tile([128, 128], f32)
        ident_r = persist.tile([128, 128], f32r)
        qsT = persist.tile([128, T], f32r)   # rows 0:64 head0, 64:128 head1
        ksT = persist.tile([128, T], f32r)
        # V blocks with an appended ones-column per head:
        # [:, blk, 0:64] = V head0, [:, blk, 64] = 1,
        # [:, blk, 65:129] = V head1, [:, blk, 129] = 1
        vaug = persist.tile([128, T // KB, 2 * (D + 1)], f32r)

        xload = ctx.enter_context(tc.tile_pool(name="xload", bufs=10))
        xtp = ctx.enter_context(tc.tile_pool(name="xtp", bufs=3))
        work = ctx.enter_context(tc.tile_pool(name="work", bufs=4))
        csp = ctx.enter_context(tc.tile_pool(name="csp", bufs=2))
        ep = ctx.enter_context(tc.tile_pool(name="ep", bufs=4))
        outp = ctx.enter_context(tc.tile_pool(name="outp", bufs=3))
        mpsum = ctx.enter_context(tc.tile_pool(name="mpsum", bufs=2,
                                               space="PSUM"))
        scp = ctx.enter_context(tc.tile_pool(name="scp", bufs=2,
                                             space="PSUM"))
        ctxp = ctx.enter_context(tc.tile_pool(name="ctxp", bufs=1,
                                              space="PSUM"))

        def load_chunk(ci):
            """Issue the DMA loads for one 512-token chunk of X."""
            tok0 = ci * TC
            tiles = []
            for tb in range(TC // 128):
                xs = xload.tile([128, DM], f32r, name=f"xs{ci}_{tb}", tag="xs")
                nc.sync.dma_start(
                    out=xs, in_=x[tok0 + tb * 128:tok0 + (tb + 1) * 128, :])
                tiles.append(xs)
            return tiles

        # --- setup, ordered so the first X chunk's DMAs go out first ---
        xs_cur = load_chunk(0)
        for hb in range(NHB):   # s-projection weights are needed first
            nc.sync.dma_start(out=w_sb[:, 3, hb, :],
                              in_=ws[hb * 128:(hb + 1) * 128, :])
        for mi, bvec in enumerate((bq, bk, bv, bs)):
            nc.sync.dma_start(out=bias_sb[:, mi:mi + 1],
                              in_=bvec[:].rearrange("(p o) -> p o", o=1))
        nc.vector.tensor_scalar(neg_bs, bias_sb[:, 3:4], -1.0, None, Alu.mult)
        make_identity(nc, ident)
        nc.vector.tensor_copy(ident_r, ident)
        # Ones columns of vaug (f32r): write exact 1.0 via (x*0)+1 from a
        # known-finite source (gpsimd/DVE memset of f32r fails the ISA check).
        ones_src = ident[:, 0:T // KB].rearrange("p (a o) -> p a o", o=1)
        nc.vector.tensor_scalar(vaug[:, :, D:D + 1], ones_src, 0.0, 1.0,
                                Alu.mult, Alu.add)
        nc.vector.tensor_scalar(vaug[:, :, 2 * D + 1:2 * D + 2], ones_src,
                                0.0, 1.0, Alu.mult, Alu.add)
        for mi, w in enumerate((wq, wk, wv)):
            for hb in range(NHB):
                nc.sync.dma_start(out=w_sb[:, mi, hb, :],
                                  in_=w[hb * 128:(hb + 1) * 128, :])

        def chunk_steps(ci, xs_tiles):
            """Generator emitting one chunk's transpose+projection work in
            small steps (yield points) so it can be interleaved."""
            tok0 = ci * TC
            tsl = slice(tok0, tok0 + TC)
            xt = xtp.tile([128, NHB, TC], f32r, name=f"xt{ci}", tag="xt")
            for tb in range(TC // 128):
                xs = xs_tiles[tb]
                for hq in range(NHB // 4):
                    # 4 transposes into one PSUM bank, then a single strided
                    # ACT copy into xt — ACT is idle during projections, and
                    # activation(Copy) does not disturb the Exp table.
                    tp = mpsum.tile([128, 512], f32r,
                                    name=f"tp{ci}_{tb}_{hq}", tag="mp")
                    for j in range(4):
                        hb = 4 * hq + j
                        nc.tensor.transpose(tp[:, j * 128:(j + 1) * 128],
                                            xs[:, hb * 128:(hb + 1) * 128],
                                            ident_r)
                    yield
                    nc.scalar.copy(
                        out=xt[:, 4 * hq:4 * hq + 4, tb * 128:(tb + 1) * 128],
                        in_=tp.rearrange("p (a b) -> p a b", a=4))
                    yield

            def mm_chain(mi, name):
                ps = mpsum.tile([128, TC], f32, name=name, tag="mp")
                for hb in range(NHB):
                    nc.tensor.matmul(ps, w_sb[:, mi, hb, :], xt[:, hb, :],
                                     start=(hb == 0), stop=(hb == NHB - 1),
                                     skip_group_check=True)
                return ps

            # s projection: sig = 1/(1+exp(-(x+bs))); sk = sig+0.5, sq = sk/8
            ps_s = mm_chain(3, f"ps_s{ci}")
            yield
            eneg = work.tile([128, TC], f32, name=f"eneg{ci}", tag="wk")
            nc.scalar.activation(eneg, ps_s, AF.Exp, bias=neg_bs, scale=-1.0)
            den = work.tile([128, TC], f32, name=f"den{ci}", tag="wk")
            nc.gpsimd.tensor_scalar(den, eneg, 1.0, None, Alu.add)
            sig = work.tile([128, TC], f32, name=f"sig{ci}", tag="wk")
            nc.vector.reciprocal(sig, den)
            yield
            sq = work.tile([128, TC], f32, name=f"sq{ci}", tag="wk")
            nc.vector.tensor_scalar(sq, sig, 0.125, 0.0625, Alu.mult, Alu.add)
            sk = work.tile([128, TC], f32, name=f"sk{ci}", tag="wk")
            nc.vector.tensor_scalar(sk, sig, 0.5, None, Alu.add)
            yield

            ps_q = mm_chain(0, f"ps_q{ci}")
            yield
            nc.vector.scalar_tensor_tensor(qsT[:, tsl], ps_q, bias_sb[:, 0:1],
                                           sq, Alu.add, Alu.mult)
            yield
            ps_k = mm_chain(1, f"ps_k{ci}")
            yield
            nc.vector.scalar_tensor_tensor(ksT[:, tsl], ps_k, bias_sb[:, 1:2],
                                           sk, Alu.add, Alu.mult)
            yield
            ps_v = mm_chain(2, f"ps_v{ci}")
            yield
            v1 = work.tile([128, TC], f32, name=f"v1{ci}", tag="wk")
            nc.vector.tensor_scalar(v1, ps_v, bias_sb[:, 2:3], None, Alu.add)
            yield
            for i in range(TC // 128):
                blk = tok0 // KB + i
                tpv = mpsum.tile([128, 128], f32, name=f"tpv{ci}_{i}",
                                 tag="mp")
                nc.tensor.transpose(tpv, v1[:, i * 128:(i + 1) * 128], ident)
                nc.vector.tensor_copy(vaug[:, blk, 0:D], tpv[:, 0:D])
                nc.vector.tensor_copy(vaug[:, blk, D + 1:2 * D + 1],
                                      tpv[:, D:2 * D])
                yield

        # ---- pumping machinery: epilogues first, then projection work ----
        epi_queue = deque()
        proj_gen = [None]

        def pump(n_epi=2, n_proj=3):
            for _ in range(n_epi):
                if not epi_queue:
                    break
                if next(epi_queue[0], "done") == "done":
                    epi_queue.popleft()
            for _ in range(n_proj):
                g = proj_gen[0]
                if g is None:
                    # spare capacity: keep draining epilogues
                    if epi_queue and next(epi_queue[0], "done") == "done":
                        epi_queue.popleft()
                    continue
                if next(g, "done") == "done":
                    proj_gen[0] = None

        def attn_epilogue_steps(b, qc, ctx0, ctx1):
            q0 = b * S + qc * QC
            cs0 = csp.tile([D + 1, QC], f32, name=f"cs0_{b}_{qc}", tag="cs0")
            cs1 = csp.tile([D + 1, QC], f32, name=f"cs1_{b}_{qc}", tag="cs1")
            nc.vector.tensor_copy(cs0, ctx0)
            nc.vector.tensor_copy(cs1, ctx1)
            yield
            for tb in range(QC // 128):
                csl = slice(tb * 128, (tb + 1) * 128)
                tp = mpsum.tile([128, 2 * (D + 1)], f32,
                                name=f"tpc{b}_{qc}_{tb}", tag="mp")
                nc.tensor.transpose(tp[:, 0:D + 1], cs0[:, csl],
                                    ident[0:D + 1, 0:D + 1])
                nc.tensor.transpose(tp[:, D + 1:2 * (D + 1)], cs1[:, csl],
                                    ident[0:D + 1, 0:D + 1])
                yield
                rec = work.tile([128, 2], f32, name=f"rec{b}_{qc}_{tb}",
                                tag="rec")
                nc.vector.reciprocal(rec[:, 0:1], tp[:, D:D + 1])
                nc.vector.reciprocal(rec[:, 1:2], tp[:, 2 * D + 1:2 * D + 2])
                ot = outp.tile([128, CW], f32, name=f"ot{b}_{qc}_{tb}",
                               tag="ot")
                nc.vector.tensor_scalar(ot[:, 0:D], tp[:, 0:D], rec[:, 0:1],
                                        None, Alu.mult)
                nc.vector.tensor_scalar(ot[:, D:2 * D],
                                        tp[:, D + 1:D + 1 + D], rec[:, 1:2],
                                        None, Alu.mult)
                nc.scalar.dma_start(
                    out=out[q0 + tb * 128:q0 + (tb + 1) * 128, :], in_=ot)
                yield

        def attn_unit(b, qc):
            """Attention for one (batch, 512-query chunk), both heads."""
            q0 = b * S + qc * QC
            qsl = slice(q0, q0 + QC)
            ctx0 = ctxp.tile([D + 1, QC], f32, name=f"ctx0_{b}_{qc}",
                             tag="ctx0")
            ctx1 = ctxp.tile([D + 1, QC], f32, name=f"ctx1_{b}_{qc}",
                             tag="ctx1")

            def sc_both(kb):
                """Both heads' scores^T into one [128, 2*QC] psum tile."""
                k0 = b * S + kb * KB
                ksl = slice(k0, k0 + KB)
                sc = scp.tile([128, 2 * QC], f32, name=f"sc_{b}_{qc}_{kb}",
                              tag="sc")
                nc.tensor.matmul(sc[:, 0:QC], ksT[0:64, ksl], qsT[0:64, qsl],
                                 start=True, stop=True, skip_group_check=True)
                nc.tensor.matmul(sc[:, QC:2 * QC], ksT[64:128, ksl],
                                 qsT[64:128, qsl],
                                 start=True, stop=True, skip_group_check=True)
                return sc

            cur = sc_both(0)
            for kb in range(NKB):
                pump()
                nxt = sc_both(kb + 1) if kb < NKB - 1 else None
                blk = b * NKB + kb
                e = ep.tile([128, 2 * QC], f32r, name=f"e_{b}_{qc}_{kb}",
                            tag="e")
                nc.scalar.activation(e, cur, AF.Exp)
                nc.tensor.matmul(ctx0, vaug[:, blk, 0:D + 1], e[:, 0:QC],
                                 start=(kb == 0), stop=(kb == NKB - 1),
                                 skip_group_check=True)
                nc.tensor.matmul(ctx1, vaug[:, blk, D + 1:2 * (D + 1)],
                                 e[:, QC:2 * QC],
                                 start=(kb == 0), stop=(kb == NKB - 1),
                                 skip_group_check=True)
                cur = nxt
            epi_queue.append(attn_epilogue_steps(b, qc, ctx0, ctx1))

        # ---- Phase 1 prologue: batch-0 chunks, software-pipelined. ----
        b0_chunks = list(range(NTC // 2))
        b1_chunks = list(range(NTC // 2, NTC))
        for i, ci in enumerate(b0_chunks):
            xs_nxt = (load_chunk(b0_chunks[i + 1])
                      if i + 1 < len(b0_chunks) else None)
            for _ in chunk_steps(ci, xs_cur):
                pass
            xs_cur = xs_nxt

        # ---- Phase 2a: batch-0 attention with batch-1 projection pumped
        # into the PE gaps (attention is ACT/exp-bound). ----
        b1_xs = {ci: load_chunk(ci) for ci in b1_chunks}

        def b1_steps():
            for ci in b1_chunks:
                yield from chunk_steps(ci, b1_xs[ci])

        proj_gen[0] = b1_steps()
        for qc in range(NQC):
            attn_unit(0, qc)
        # Batch-1 attention depends on all of batch-1's projections: the
        # remaining projection work MUST be emitted before phase 2b.
        while proj_gen[0] is not None:
            if next(proj_gen[0], "done") == "done":
                proj_gen[0] = None

        # ---- Phase 2b: batch-1 attention. ----
        for qc in range(NQC):
            attn_unit(1, qc)
        while epi_queue:
            if next(epi_queue[0], "done") == "done":
                epi_queue.popleft()

    nc.compile()
    return nc


def _get_nc():
    if "nc" not in _CACHE:
        _CACHE["nc"] = _build_nc()
    return _CACHE["nc"]


def _shard_inputs(hidden_states, Wq, bq, Wk, bk, Wv, bv, Ws, bs):
    x2d = np.ascontiguousarray(
        np.asarray(hidden_states, dtype=np.float32).reshape(T, DM))
    in_maps = []
    for c in range(N_CORES):
        cols = slice(c * CW, (c + 1) * CW)
        in_maps.append({
            "x": x2d,
            "wq": np.ascontiguousarray(np.asarray(Wq, np.float32)[:, cols]),
            "wk": np.ascontiguousarray(np.asarray(Wk, np.float32)[:, cols]),
            "wv": np.ascontiguousarray(np.asarray(Wv, np.float32)[:, cols]),
            "ws": np.ascontiguousarray(np.asarray(Ws, np.float32)[:, cols]),
            "bq": np.ascontiguousarray(np.asarray(bq, np.float32)[cols]),
            "bk": np.ascontiguousarray(np.asarray(bk, np.float32)[cols]),
            "bv": np.ascontiguousarray(np.asarray(bv, np.float32)[cols]),
            "bs": np.ascontiguousarray(np.asarray(bs, np.float32)[cols]),
        })
    return in_maps


def _assemble(results):
    full = np.empty((T, DM), np.float32)
    for c in range(N_CORES):
        full[:, c * CW:(c + 1) * CW] = results[c]["out"]
    return full.reshape(B, S, DM)


def kernel(hidden_states, Wq, bq, Wk, bk, Wv, bv, Ws, bs):
    from concourse.bass_utils import run_bass_kernel_spmd

    nc = _get_nc()
    in_maps = _shard_inputs(hidden_states, Wq, bq, Wk, bk, Wv, bv, Ws, bs)
    res = run_bass_kernel_spmd(nc, in_maps, list(range(N_CORES)))
    return _assemble(res.results)
